# revision 61
# baseline (speedup 1.0000x reference)
"""Multi-head self-attention (B=2, T=2048, C=1024, H=16) on 8 TRN2 NeuronCores.

Sharding: hybrid batch x head-group. Core c owns batch b = c//4 and heads
4g..4g+3 where g = c%4 (two head-pairs). Per core:
  - QKV projection for its batch rows and its 4 heads (bf16 matmuls,
    contraction-major x^T layout, biases folded in via tensor_scalar_add)
  - causal attention for its 4 heads, flash-style blockwise exp with no
    max-subtraction (scores are O(1)); causal mask applied as a -1e9 bias
    accumulated into the scores PSUM via an identity matmul, so exp gives
    exact zeros (no separate mask multiply)
  - softmax denominator via a ones-column in the value matrix (row 64 of
    the AV accumulation); per-i-chunk normalization with
    reciprocal_approx_fast + a selector-matmul broadcast
  - partial output projection partial = values @ Wo[rows of its heads]
Host sums the 4 partials per batch and adds bo.
"""

import numpy as np
import ml_dtypes

import concourse.bass as bass
import concourse.bacc as bacc
import concourse.mybir as mybir
import concourse.tile as tile
from concourse.bass_utils import run_bass_kernel_spmd

B, T, C = 2, 2048, 1024
H, HS = 16, 64
N_CORES = 8
IC_W = 512                     # i-chunk width (query cols per block)
N_IC = T // IC_W               # 4 i-chunks
JT = 128                       # j-tile width (key rows per block)
N_JT = T // JT                 # 16 j-tiles
BF16 = mybir.dt.bfloat16
F32 = mybir.dt.float32
NPBF = ml_dtypes.bfloat16


def _build(causal: bool):
    nc = bacc.Bacc("TRN2", target_bir_lowering=False, debug=False,
                   num_devices=N_CORES)

    xt_d = nc.dram_tensor("xt", [N_IC, 128, 8 * IC_W], BF16,
                          kind="ExternalInput")
    w3_d = nc.dram_tensor("w3", [128, 8 * 768], BF16, kind="ExternalInput")
    b3_d = nc.dram_tensor("b3", [128, 6], F32, kind="ExternalInput")
    wo_d = nc.dram_tensor("wo", [128, 2 * C], BF16, kind="ExternalInput")
    cst_d = nc.dram_tensor("cst", [128, 448], BF16, kind="ExternalInput")
    vni_d = nc.dram_tensor("vni", [128, 4 * 65 * N_JT], BF16,
                           kind="ExternalInput")
    out_d = nc.dram_tensor("part", [T, C], BF16, kind="ExternalOutput")

    with tile.TileContext(nc) as tc:
        with (
            tc.tile_pool(name="const", bufs=1) as cpool,
            tc.tile_pool(name="xt", bufs=4) as xtpool,
            tc.tile_pool(name="pt", bufs=6) as ptpool,
            tc.tile_pool(name="otu", bufs=4) as otupool,
            tc.tile_pool(name="ot", bufs=4) as otpool,
            tc.tile_pool(name="csr", bufs=8) as csrpool,
            tc.tile_pool(name="rt", bufs=2) as rtpool,
            tc.tile_pool(name="osb", bufs=3) as opool,
            tc.tile_pool(name="ps_s", bufs=2, space="PSUM") as ps_s,
            tc.tile_pool(name="ps_o", bufs=2, space="PSUM") as ps_o,
            tc.tile_pool(name="ps_m", bufs=2, space="PSUM") as ps_m,
        ):
            # ---- constants (contiguous host-blocked DMAs; x chunks early) --
            xts_all = []
            xtts = []
            for icl in range(N_IC):
                xtt = xtpool.tile([128, 8 * IC_W], BF16, tag="xt", name="xtt")
                xtts.append(xtt)
                xts_all.append([xtt[:, IC_W * ct:IC_W * (ct + 1)]
                                for ct in range(8)])
            # first chunk + weights land per-ct so the first QKV
            # accumulation chain starts as soon as block 0 arrives
            w3t = cpool.tile([128, 8 * 768], BF16, tag="w3")
            w3_sb = [w3t[:, 768 * ct:768 * (ct + 1)] for ct in range(8)]
            b3t = cpool.tile([128, 6], F32, tag="b3")
            nc.sync.dma_start(b3t[:], b3_d.ap()[:])
            for ct in range(8):
                nc.sync.dma_start(xts_all[0][ct],
                                  xt_d.ap()[0][:, IC_W * ct:IC_W * (ct + 1)])
                nc.scalar.dma_start(w3_sb[ct],
                                    w3_d.ap()[:, 768 * ct:768 * (ct + 1)])
            b3_sb = [b3t[:, i:i + 1] for i in range(6)]
            for icl in range(1, N_IC):
                nc.scalar.dma_start(xtts[icl][:], xt_d.ap()[icl])
            wot = cpool.tile([128, 2 * C], BF16, tag="wo")
            nc.sync.dma_start(wot[:], wo_d.ap()[:])
            wo_sb = [wot[:, C * p:C * (p + 1)] for p in range(2)]
            cst = cpool.tile([128, 448], BF16, tag="cst")
            nc.sync.dma_start(cst[:], cst_d.ap()[:])
            msk01 = cst[:, 0:128]
            idn_sb = cst[:, 128:256]
            onesr = cst[0:1, 256:320]
            mskneg = cst[:, 320:448]
            vnt = cpool.tile([128, 4 * 65 * N_JT], BF16, tag="vn")
            nc.sync.dma_start(vnt[:], vni_d.ap()[:])
            vn_sb = [vnt[:, 65 * N_JT * h:65 * N_JT * (h + 1)]
                     for h in range(4)]

            qt2 = [cpool.tile([128, T], BF16, tag=f"qt2_{p}", name=f"qt2_{p}")
                   for p in range(2)]
            kt2 = [cpool.tile([128, T], BF16, tag=f"kt2_{p}", name=f"kt2_{p}")
                   for p in range(2)]
            vt2 = [cpool.tile([128, T], BF16, tag=f"vt2_{p}", name=f"vt2_{p}")
                   for p in range(2)]


            def emit_qkv(icl):
                i0 = IC_W * icl
                xts = xts_all[icl]
                for p in range(2):
                    for n, dsts in enumerate((qt2, kt2, vt2)):
                        c0 = 384 * p + 128 * n
                        ps = ps_m.tile([128, IC_W], F32, tag="m", name="ps_qkv")
                        for ct in range(8):
                            nc.tensor.matmul(
                                ps[:], w3_sb[ct][:, c0:c0 + 128], xts[ct],
                                start=(ct == 0), stop=(ct == 7))
                        nc.vector.tensor_scalar_add(
                            dsts[p][:, i0:i0 + IC_W], ps[:],
                            b3_sb[3 * p + n][:])

            def emit_vtrans(icl):
                for p in range(2):
                    for jl in range(4):
                        jt = 4 * icl + jl
                        j0 = JT * jt
                        psv = ps_m.tile([128, JT], BF16, tag="m", name="psv")
                        nc.tensor.transpose(
                            psv[:, 0:JT], vt2[p][:, j0:j0 + JT], idn_sb)
                        nc.vector.tensor_copy(
                            vn_sb[2 * p][:, 65 * jt:65 * jt + 64],
                            psv[:, 0:64])
                        nc.vector.tensor_copy(
                            vn_sb[2 * p + 1][:, 65 * jt:65 * jt + 64],
                            psv[:, 64:128])

            def emit_attention_p(icl, p):
                i0 = IC_W * icl
                njt = 4 * (icl + 1) if causal else N_JT
                pso = [ps_o.tile([128, IC_W], F32, tag="o",
                                 name=f"pso{h_}") for h_ in range(2)]
                for jtp in range(njt // 2):
                    pts = []
                    for h in range(2):
                        st = ps_s.tile([128, 2 * IC_W], F32, tag="s",
                                       name="st")
                        for half in range(2):
                            jt = 2 * jtp + half
                            j0 = JT * jt
                            c0 = IC_W * half
                            # diagonal tiles: only i >= 128*r_ is live
                            r_ = jt - 4 * icl if causal else -1
                            v0 = JT * r_ if r_ > 0 else 0
                            nc.tensor.matmul(
                                st[:, c0 + v0:c0 + IC_W],
                                kt2[p][64 * h:64 * h + 64, j0:j0 + JT],
                                qt2[p][64 * h:64 * h + 64,
                                       i0 + v0:i0 + IC_W],
                                start=True, stop=True,
                                tile_position=(64 * h, 0))
                        pt = ptpool.tile([128, 2 * IC_W], BF16, tag="pt",
                                         name="pt")
                        if causal and 2 * jtp >= 4 * icl:
                            # diagonal pair: exp only the live staircase
                            for half in range(2):
                                jt = 2 * jtp + half
                                c0 = IC_W * half
                                v0 = JT * (jt - 4 * icl)
                                nc.scalar.activation(
                                    pt[:, c0 + v0:c0 + IC_W],
                                    st[:, c0 + v0:c0 + IC_W],
                                    mybir.ActivationFunctionType.Exp)
                                # zero the masked triangle boundary block
                                nc.vector.tensor_mul(
                                    pt[:, c0 + v0:c0 + v0 + JT],
                                    pt[:, c0 + v0:c0 + v0 + JT], msk01)
                        else:
                            nc.scalar.activation(
                                pt[:], st[:],
                                mybir.ActivationFunctionType.Exp)
                        pts.append(pt)
                    for h in range(2):
                        for half in range(2):
                            jt = 2 * jtp + half
                            c0 = IC_W * half
                            r_ = jt - 4 * icl if causal else -1
                            v0 = JT * r_ if r_ > 0 else 0
                            nc.tensor.matmul(
                                pso[h][0:65, v0:IC_W],
                                vn_sb[2 * p + h][:, 65 * jt:65 * jt + 65],
                                pts[h][:, c0 + v0:c0 + IC_W],
                                start=(jt == 0), stop=(jt == njt - 1),
                                tile_position=(0, 0), skip_group_check=True)
                # unnormalized values + denominator rows out of PSUM
                otu = otupool.tile([128, IC_W], BF16, tag="otu", name="otu")
                csrs = []
                for h in range(2):
                    csr = csrpool.tile([1, IC_W], BF16, tag="csr", name="csr")
                    nc.vector.tensor_copy(csr[:], pso[h][64:65, :])
                    nc.vector.tensor_copy(
                        otu[64 * h:64 * h + 64, :], pso[h][0:64, :])
                    csrs.append(csr)
                return otu, csrs

            def emit_norm(otus):
                ots = []
                for p in range(2):
                    otu, csrs = otus[p]
                    dn = ps_m.tile([128, IC_W], F32, tag="m", name="dn")
                    nc.tensor.matmul(dn[0:64, :], onesr, csrs[0][:],
                                     start=True, stop=True)
                    nc.tensor.matmul(dn[64:128, :], onesr, csrs[1][:],
                                     start=True, stop=True)
                    rt = rtpool.tile([128, IC_W], F32, tag="rt", name="rt")
                    nc.vector.reciprocal_approx_fast(rt[:], dn[:])
                    ot = otpool.tile([128, IC_W], BF16, tag="ot", name="ot")
                    nc.vector.tensor_mul(ot[:], otu[:], rt[:])
                    ots.append(ot)
                return ots

            def emit_outproj(icl, ots):
                i0 = IC_W * icl
                for it in range(4):
                    osb = opool.tile([128, C], BF16, tag="osb", name="osb")
                    for ch in range(2):
                        psp = ps_m.tile([128, IC_W], F32, tag="m", name="psp")
                        nc.tensor.matmul(
                            psp[:], ots[0][:, 128 * it:128 * (it + 1)],
                            wo_sb[0][:, IC_W * ch:IC_W * (ch + 1)],
                            start=True, stop=False)
                        nc.tensor.matmul(
                            psp[:], ots[1][:, 128 * it:128 * (it + 1)],
                            wo_sb[1][:, IC_W * ch:IC_W * (ch + 1)],
                            start=False, stop=True)
                        # alternate evacuation engines so the psum ring
                        # drains as fast as the projection matmuls fill it
                        if ch == 0:
                            nc.vector.tensor_copy(
                                osb[:, IC_W * ch:IC_W * (ch + 1)], psp[:])
                        else:
                            nc.scalar.activation(
                                osb[:, IC_W * ch:IC_W * (ch + 1)], psp[:],
                                mybir.ActivationFunctionType.Copy)
                    r0 = i0 + 128 * it
                    nc.sync.dma_start(out_d.ap()[r0:r0 + 128, :], osb[:])

            # software pipeline: chunk icl's normalize + projection PE work
            # is emitted inside chunk icl+1's stream so the in-order PE
            # queue never stalls on the DVE normalize chain.
            pend_otus, pend_ots = None, None
            for icl in range(N_IC):
                emit_qkv(icl)
                emit_vtrans(icl)
                if pend_otus is not None:
                    pend_ots = emit_norm(pend_otus)
                o0 = emit_attention_p(icl, 0)
                if pend_ots is not None:
                    emit_outproj(icl - 1, pend_ots)
                    pend_ots = None
                o1 = emit_attention_p(icl, 1)
                pend_otus = [o0, o1]
            pend_ots = emit_norm(pend_otus)
            emit_outproj(N_IC - 1, pend_ots)
    nc.compile()
    return nc


_PROGS = {}


def _get_prog(causal: bool):
    if causal not in _PROGS:
        _PROGS[causal] = _build(causal)
    return _PROGS[causal]


def _prep_inputs(x, Wqkv, bqkv, Wo):
    """Per-core input maps (host-side sharding)."""
    x = np.asarray(x, dtype=np.float32)
    Wqkv = np.asarray(Wqkv, dtype=np.float32)
    bqkv = np.asarray(bqkv, dtype=np.float32)
    Wo = np.asarray(Wo, dtype=np.float32)

    scale = 1.0 / np.sqrt(np.float32(HS))

    jl = np.arange(JT)[:, None]
    il = np.arange(JT)[None, :]
    cst = np.zeros((128, 448), dtype=NPBF)
    cst[:, 0:128] = np.where(jl <= il, 1.0, 0.0).astype(NPBF)
    cst[:, 128:256] = np.eye(128, dtype=NPBF)
    cst[0, 256:320] = 1
    cst[:, 320:448] = np.where(jl <= il, 0.0, -1e9).astype(NPBF)
    vni = np.zeros((128, 4 * 65 * N_JT), dtype=NPBF)
    vni[:, 64::65] = 1

    # block x^T per i-chunk: [N_IC, 128, 8ct*512] so DMAs are contiguous
    xts = [np.ascontiguousarray(
        x[b].T.astype(NPBF).reshape(8, 128, N_IC, IC_W)
        .transpose(2, 1, 0, 3).reshape(N_IC, 128, 8 * IC_W))
        for b in range(B)]

    in_maps = []
    for c in range(N_CORES):
        b, g = c // 4, c % 4
        heads = [4 * g + k for k in range(4)]
        w3_cols, b3_rows = [], []
        for p in range(2):
            pair = heads[2 * p:2 * p + 2]
            for off, sc in ((0, scale), (HS, 1.0), (2 * HS, 1.0)):
                w3_cols.append(np.concatenate(
                    [Wqkv[:, 192 * h + off:192 * h + off + HS] * sc
                     for h in pair], axis=1))
                b3_rows.append(np.concatenate(
                    [bqkv[192 * h + off:192 * h + off + HS] * sc
                     for h in pair]))
        w3 = np.concatenate(w3_cols, axis=1).astype(NPBF)       # [1024, 768]
        w3b = w3.reshape(8, 128, 768).transpose(1, 0, 2).reshape(128, 8 * 768)
        b3 = np.stack(b3_rows).astype(np.float32)               # [6, 128]
        wo = np.concatenate([Wo[HS * h:HS * (h + 1), :] for h in heads],
                            axis=0).astype(NPBF)                # [256, 1024]
        wob = wo.reshape(2, 128, C).transpose(1, 0, 2).reshape(128, 2 * C)
        in_maps.append({
            "xt": xts[b],
            "w3": np.ascontiguousarray(w3b),
            "b3": np.ascontiguousarray(b3.T),
            "wo": np.ascontiguousarray(wob),
            "cst": cst,
            "vni": vni,
        })
    return in_maps


class _Runner:
    """Cached shard_map runner for the SPMD NEFF (avoids re-jit per call)."""

    def __init__(self, nc):
        import jax
        from jax.sharding import Mesh, PartitionSpec
        from jax.experimental.shard_map import shard_map
        from concourse import bass2jax

        bass2jax.install_neuronx_cc_hook()

        part_name = (nc.partition_id_tensor.name
                     if nc.partition_id_tensor else None)
        in_names, out_names, out_avals, zero_outs = [], [], [], []
        for alloc in nc.m.functions[0].allocations:
            if not isinstance(alloc, mybir.MemoryLocationSet):
                continue
            name = alloc.memorylocations[0].name
            if alloc.kind == "ExternalInput":
                if name != part_name:
                    in_names.append(name)
            elif alloc.kind == "ExternalOutput":
                out_names.append(name)
                shape = tuple(alloc.tensor_shape)
                dtype = mybir.dt.np(alloc.dtype)
                out_avals.append(jax.core.ShapedArray(shape, dtype))
                zero_outs.append(np.zeros(shape, dtype))
        self.in_names, self.out_names = in_names, out_names
        self.zero_outs = zero_outs
        n_params, n_outs = len(in_names), len(out_names)
        all_in_names = tuple(in_names) + tuple(out_names)
        if part_name is not None:
            all_in_names = all_in_names + (part_name,)

        def _exec(args, outs):
            operands = list(args) + list(outs)
            if part_name is not None:
                operands.append(bass2jax.partition_id_tensor())
            return bass2jax._bass_exec_p.bind(
                *operands,
                out_avals=tuple(out_avals),
                in_names=all_in_names,
                out_names=tuple(out_names),
                lowering_input_output_aliases=(),
                sim_require_finite=True,
                sim_require_nnan=True,
                nc=nc)

        def _body(*args):
            ins, outs = args[:n_params], list(args[n_params:])
            return tuple(_exec(ins, outs))

        devices = jax.devices()[:N_CORES]
        mesh = Mesh(np.asarray(devices), ("core",))
        donate = tuple(range(n_params, n_params + n_outs))
        self._fn = jax.jit(
            shard_map(_body, mesh=mesh,
                      in_specs=(PartitionSpec("core"),) * (n_params + n_outs),
                      out_specs=(PartitionSpec("core"),) * n_outs,
                      check_rep=False),
            donate_argnums=donate, keep_unused=True)

    def __call__(self, in_maps):
        concat_in = [
            np.concatenate([in_maps[c][k] for c in range(N_CORES)], axis=0)
            for k in self.in_names]
        concat_zero = [
            np.zeros((N_CORES * z.shape[0], *z.shape[1:]), z.dtype)
            for z in self.zero_outs]
        out = self._fn(*concat_in, *concat_zero)
        return [
            {k: np.asarray(out[i]).reshape(N_CORES, *self.zero_outs[i].shape)[c]
             for i, k in enumerate(self.out_names)}
            for c in range(N_CORES)]


_RUNNERS = {}


def _get_runner(causal: bool):
    if causal not in _RUNNERS:
        _RUNNERS[causal] = _Runner(_get_prog(causal))
    return _RUNNERS[causal]


def kernel(x, Wqkv, bqkv, Wo, bo, mask):
    causal = bool(np.asarray(mask).item()) if not isinstance(mask, (int, bool)) \
        else bool(mask)
    runner = _get_runner(causal)
    in_maps = _prep_inputs(x, Wqkv, bqkv, Wo)
    results = runner(in_maps)
    out = np.zeros((B, T, C), dtype=np.float32)
    for c in range(N_CORES):
        out[c // 4] += results[c]["part"].astype(np.float32)
    out += np.asarray(bo, dtype=np.float32)[None, None, :]
    return out


# revision 63
# speedup vs baseline: 1.0587x; 1.0587x over previous
"""Multi-head self-attention (B=2, T=2048, C=1024, H=16) on 8 TRN2 NeuronCores.

Sharding: hybrid batch x head-group. Core c owns batch b = c//4 and heads
4g..4g+3 where g = c%4 (two head-pairs). Per core:
  - QKV projection for its batch rows and its 4 heads (bf16 matmuls,
    contraction-major x^T layout, biases folded in via tensor_scalar_add)
  - causal attention for its 4 heads, flash-style blockwise exp with no
    max-subtraction (scores are O(1)); diagonal tiles run partial-width
    (scores/exp/AV only touch the live staircase) with a 0/1 triangle
    multiply on the 128-wide boundary block
  - softmax denominator via a ones-column in the value matrix (row 64 of
    the AV accumulation); per-i-chunk normalization with
    reciprocal_approx_fast + a ones-matmul broadcast
  - partial output projection partial = values @ Wo[rows of its heads],
    software-pipelined into the next chunk's stream so the in-order PE
    queue never stalls (keeps the HAM clock-gate at full speed)
Host sums the 4 partials per batch and adds bo.
"""

import numpy as np
import ml_dtypes

import concourse.bass as bass
import concourse.bacc as bacc
import concourse.mybir as mybir
import concourse.tile as tile
from concourse.bass_utils import run_bass_kernel_spmd

B, T, C = 2, 2048, 1024
H, HS = 16, 64
N_CORES = 8
IC_W = 512                     # i-chunk width (query cols per block)
N_IC = T // IC_W               # 4 i-chunks
JT = 128                       # j-tile width (key rows per block)
N_JT = T // JT                 # 16 j-tiles
BF16 = mybir.dt.bfloat16
F32 = mybir.dt.float32
NPBF = ml_dtypes.bfloat16


def _build(causal: bool):
    nc = bacc.Bacc("TRN2", target_bir_lowering=False, debug=False,
                   num_devices=N_CORES)

    xt_d = nc.dram_tensor("xt", [N_IC, 128, 8 * IC_W], BF16,
                          kind="ExternalInput")
    w3_d = nc.dram_tensor("w3", [128, 8 * 768], BF16, kind="ExternalInput")
    b3_d = nc.dram_tensor("b3", [128, 6], F32, kind="ExternalInput")
    wo_d = nc.dram_tensor("wo", [128, 2 * C], BF16, kind="ExternalInput")
    cst_d = nc.dram_tensor("cst", [128, 448], BF16, kind="ExternalInput")
    vni_d = nc.dram_tensor("vni", [128, 4 * 65 * N_JT], BF16,
                           kind="ExternalInput")
    out_d = nc.dram_tensor("part", [T, C], BF16, kind="ExternalOutput")

    with tile.TileContext(nc) as tc:
        with (
            tc.tile_pool(name="const", bufs=1) as cpool,
            tc.tile_pool(name="xt", bufs=4) as xtpool,
            tc.tile_pool(name="pt", bufs=4) as ptpool,
            tc.tile_pool(name="otu", bufs=4) as otupool,
            tc.tile_pool(name="ot", bufs=4) as otpool,
            tc.tile_pool(name="csr", bufs=8) as csrpool,
            tc.tile_pool(name="rt", bufs=2) as rtpool,
            tc.tile_pool(name="osb", bufs=2) as opool,
            tc.tile_pool(name="ps_s", bufs=2, space="PSUM") as ps_s,
            tc.tile_pool(name="ps_o", bufs=2, space="PSUM") as ps_o,
            tc.tile_pool(name="ps_m", bufs=2, space="PSUM") as ps_m,
        ):
            # ---- constants (contiguous host-blocked DMAs; x chunks early) --
            xts_all = []
            xtts = []
            for icl in range(N_IC):
                xtt = xtpool.tile([128, 8 * IC_W], BF16, tag="xt", name="xtt")
                xtts.append(xtt)
                xts_all.append([xtt[:, IC_W * ct:IC_W * (ct + 1)]
                                for ct in range(8)])
            # first chunk + weights land per-ct so the first QKV
            # accumulation chain starts as soon as block 0 arrives
            w3t = cpool.tile([128, 8 * 768], BF16, tag="w3")
            w3_sb = [w3t[:, 768 * ct:768 * (ct + 1)] for ct in range(8)]
            b3t = cpool.tile([128, 6], F32, tag="b3")
            nc.sync.dma_start(b3t[:], b3_d.ap()[:])
            for ct in range(8):
                nc.sync.dma_start(xts_all[0][ct],
                                  xt_d.ap()[0][:, IC_W * ct:IC_W * (ct + 1)])
                nc.scalar.dma_start(w3_sb[ct],
                                    w3_d.ap()[:, 768 * ct:768 * (ct + 1)])
            b3_sb = [b3t[:, i:i + 1] for i in range(6)]
            for icl in range(1, N_IC):
                nc.scalar.dma_start(xtts[icl][:], xt_d.ap()[icl])
            wot = cpool.tile([128, 2 * C], BF16, tag="wo")
            nc.sync.dma_start(wot[:], wo_d.ap()[:])
            wo_sb = [wot[:, C * p:C * (p + 1)] for p in range(2)]
            cst = cpool.tile([128, 448], BF16, tag="cst")
            nc.sync.dma_start(cst[:], cst_d.ap()[:])
            msk01 = cst[:, 0:128]
            idn_sb = cst[:, 128:256]
            onesr = cst[0:1, 256:320]
            mskneg = cst[:, 320:448]
            vnt = cpool.tile([128, 4 * 65 * N_JT], BF16, tag="vn")
            nc.sync.dma_start(vnt[:], vni_d.ap()[:])
            vn_sb = [vnt[:, 65 * N_JT * h:65 * N_JT * (h + 1)]
                     for h in range(4)]

            qt2 = [cpool.tile([128, T], BF16, tag=f"qt2_{p}", name=f"qt2_{p}")
                   for p in range(2)]
            kt2 = [cpool.tile([128, T], BF16, tag=f"kt2_{p}", name=f"kt2_{p}")
                   for p in range(2)]
            vt2 = [cpool.tile([128, T], BF16, tag=f"vt2_{p}", name=f"vt2_{p}")
                   for p in range(2)]


            def emit_qkv(icl):
                i0 = IC_W * icl
                xts = xts_all[icl]
                for p in range(2):
                    for n, dsts in enumerate((qt2, kt2, vt2)):
                        c0 = 384 * p + 128 * n
                        ps = ps_m.tile([128, IC_W], F32, tag="m", name="ps_qkv")
                        for ct in range(8):
                            nc.tensor.matmul(
                                ps[:], w3_sb[ct][:, c0:c0 + 128], xts[ct],
                                start=(ct == 0), stop=(ct == 7))
                        nc.vector.tensor_scalar_add(
                            dsts[p][:, i0:i0 + IC_W], ps[:],
                            b3_sb[3 * p + n][:])

            def emit_vtrans(icl):
                for p in range(2):
                    for jl in range(4):
                        jt = 4 * icl + jl
                        j0 = JT * jt
                        psv = ps_m.tile([128, JT], BF16, tag="m", name="psv")
                        nc.tensor.transpose(
                            psv[:, 0:JT], vt2[p][:, j0:j0 + JT], idn_sb)
                        nc.vector.tensor_copy(
                            vn_sb[2 * p][:, 65 * jt:65 * jt + 64],
                            psv[:, 0:64])
                        nc.vector.tensor_copy(
                            vn_sb[2 * p + 1][:, 65 * jt:65 * jt + 64],
                            psv[:, 64:128])

            def emit_attention_p(icl, p):
                i0 = IC_W * icl
                njt = 4 * (icl + 1) if causal else N_JT
                pso = [ps_o.tile([128, IC_W], F32, tag="o",
                                 name=f"pso{h_}") for h_ in range(2)]
                for jtp in range(njt // 2):
                    pts = []
                    for h in range(2):
                        st = ps_s.tile([128, 2 * IC_W], F32, tag="s",
                                       name="st")
                        for half in range(2):
                            jt = 2 * jtp + half
                            j0 = JT * jt
                            c0 = IC_W * half
                            # diagonal tiles: only i >= 128*r_ is live
                            r_ = jt - 4 * icl if causal else -1
                            v0 = JT * r_ if r_ > 0 else 0
                            nc.tensor.matmul(
                                st[:, c0 + v0:c0 + IC_W],
                                kt2[p][64 * h:64 * h + 64, j0:j0 + JT],
                                qt2[p][64 * h:64 * h + 64,
                                       i0 + v0:i0 + IC_W],
                                start=True, stop=True,
                                tile_position=(64 * h, 0))
                        pt = ptpool.tile([128, 2 * IC_W], BF16, tag="pt",
                                         name="pt")
                        if causal and 2 * jtp >= 4 * icl:
                            # diagonal pair: exp only the live staircase
                            for half in range(2):
                                jt = 2 * jtp + half
                                c0 = IC_W * half
                                v0 = JT * (jt - 4 * icl)
                                nc.scalar.activation(
                                    pt[:, c0 + v0:c0 + IC_W],
                                    st[:, c0 + v0:c0 + IC_W],
                                    mybir.ActivationFunctionType.Exp)
                                # zero the masked triangle boundary block
                                nc.vector.tensor_mul(
                                    pt[:, c0 + v0:c0 + v0 + JT],
                                    pt[:, c0 + v0:c0 + v0 + JT], msk01)
                        else:
                            nc.scalar.activation(
                                pt[:], st[:],
                                mybir.ActivationFunctionType.Exp)
                        pts.append(pt)
                    for h in range(2):
                        for half in range(2):
                            jt = 2 * jtp + half
                            c0 = IC_W * half
                            r_ = jt - 4 * icl if causal else -1
                            v0 = JT * r_ if r_ > 0 else 0
                            nc.tensor.matmul(
                                pso[h][0:65, v0:IC_W],
                                vn_sb[2 * p + h][:, 65 * jt:65 * jt + 65],
                                pts[h][:, c0 + v0:c0 + IC_W],
                                start=(jt == 0), stop=(jt == njt - 1),
                                tile_position=(0, 0), skip_group_check=True)
                # unnormalized values + denominator rows out of PSUM
                otu = otupool.tile([128, IC_W], BF16, tag="otu", name="otu")
                csrs = []
                for h in range(2):
                    csr = csrpool.tile([1, IC_W], BF16, tag="csr", name="csr")
                    nc.vector.tensor_copy(csr[:], pso[h][64:65, :])
                    nc.vector.tensor_copy(
                        otu[64 * h:64 * h + 64, :], pso[h][0:64, :])
                    csrs.append(csr)
                return otu, csrs

            def emit_norm(otus):
                ots = []
                for p in range(2):
                    otu, csrs = otus[p]
                    dn = ps_m.tile([128, IC_W], F32, tag="m", name="dn")
                    nc.tensor.matmul(dn[0:64, :], onesr, csrs[0][:],
                                     start=True, stop=True)
                    nc.tensor.matmul(dn[64:128, :], onesr, csrs[1][:],
                                     start=True, stop=True)
                    rt = rtpool.tile([128, IC_W], F32, tag="rt", name="rt")
                    nc.vector.reciprocal_approx_fast(rt[:], dn[:])
                    ot = otpool.tile([128, IC_W], BF16, tag="ot", name="ot")
                    nc.vector.tensor_mul(ot[:], otu[:], rt[:])
                    ots.append(ot)
                return ots

            def emit_outproj(icl, ots):
                i0 = IC_W * icl
                for it in range(4):
                    osb = opool.tile([128, C], BF16, tag="osb", name="osb")
                    for ch in range(2):
                        psp = ps_m.tile([128, IC_W], F32, tag="m", name="psp")
                        nc.tensor.matmul(
                            psp[:], ots[0][:, 128 * it:128 * (it + 1)],
                            wo_sb[0][:, IC_W * ch:IC_W * (ch + 1)],
                            start=True, stop=False)
                        nc.tensor.matmul(
                            psp[:], ots[1][:, 128 * it:128 * (it + 1)],
                            wo_sb[1][:, IC_W * ch:IC_W * (ch + 1)],
                            start=False, stop=True)
                        # alternate evacuation engines so the psum ring
                        # drains as fast as the projection matmuls fill it
                        if ch == 0:
                            nc.vector.tensor_copy(
                                osb[:, IC_W * ch:IC_W * (ch + 1)], psp[:])
                        else:
                            nc.scalar.activation(
                                osb[:, IC_W * ch:IC_W * (ch + 1)], psp[:],
                                mybir.ActivationFunctionType.Copy)
                    r0 = i0 + 128 * it
                    nc.sync.dma_start(out_d.ap()[r0:r0 + 128, :], osb[:])

            # software pipeline: chunk icl's normalize + projection PE work
            # is emitted inside chunk icl+1's stream so the in-order PE
            # queue never stalls on the DVE normalize chain.
            pend_otus, pend_ots = None, None
            for icl in range(N_IC):
                emit_qkv(icl)
                emit_vtrans(icl)
                if pend_otus is not None:
                    pend_ots = emit_norm(pend_otus)
                o0 = emit_attention_p(icl, 0)
                if pend_ots is not None:
                    emit_outproj(icl - 1, pend_ots)
                    pend_ots = None
                o1 = emit_attention_p(icl, 1)
                pend_otus = [o0, o1]
            pend_ots = emit_norm(pend_otus)
            emit_outproj(N_IC - 1, pend_ots)
    nc.compile()
    return nc


_PROGS = {}


def _get_prog(causal: bool):
    if causal not in _PROGS:
        _PROGS[causal] = _build(causal)
    return _PROGS[causal]


def _prep_inputs(x, Wqkv, bqkv, Wo):
    """Per-core input maps (host-side sharding)."""
    x = np.asarray(x, dtype=np.float32)
    Wqkv = np.asarray(Wqkv, dtype=np.float32)
    bqkv = np.asarray(bqkv, dtype=np.float32)
    Wo = np.asarray(Wo, dtype=np.float32)

    scale = 1.0 / np.sqrt(np.float32(HS))

    jl = np.arange(JT)[:, None]
    il = np.arange(JT)[None, :]
    cst = np.zeros((128, 448), dtype=NPBF)
    cst[:, 0:128] = np.where(jl <= il, 1.0, 0.0).astype(NPBF)
    cst[:, 128:256] = np.eye(128, dtype=NPBF)
    cst[0, 256:320] = 1
    cst[:, 320:448] = np.where(jl <= il, 0.0, -1e9).astype(NPBF)
    vni = np.zeros((128, 4 * 65 * N_JT), dtype=NPBF)
    vni[:, 64::65] = 1

    # block x^T per i-chunk: [N_IC, 128, 8ct*512] so DMAs are contiguous
    xts = [np.ascontiguousarray(
        x[b].T.astype(NPBF).reshape(8, 128, N_IC, IC_W)
        .transpose(2, 1, 0, 3).reshape(N_IC, 128, 8 * IC_W))
        for b in range(B)]

    in_maps = []
    for c in range(N_CORES):
        b, g = c // 4, c % 4
        heads = [4 * g + k for k in range(4)]
        w3_cols, b3_rows = [], []
        for p in range(2):
            pair = heads[2 * p:2 * p + 2]
            for off, sc in ((0, scale), (HS, 1.0), (2 * HS, 1.0)):
                w3_cols.append(np.concatenate(
                    [Wqkv[:, 192 * h + off:192 * h + off + HS] * sc
                     for h in pair], axis=1))
                b3_rows.append(np.concatenate(
                    [bqkv[192 * h + off:192 * h + off + HS] * sc
                     for h in pair]))
        w3 = np.concatenate(w3_cols, axis=1).astype(NPBF)       # [1024, 768]
        w3b = w3.reshape(8, 128, 768).transpose(1, 0, 2).reshape(128, 8 * 768)
        b3 = np.stack(b3_rows).astype(np.float32)               # [6, 128]
        wo = np.concatenate([Wo[HS * h:HS * (h + 1), :] for h in heads],
                            axis=0).astype(NPBF)                # [256, 1024]
        wob = wo.reshape(2, 128, C).transpose(1, 0, 2).reshape(128, 2 * C)
        in_maps.append({
            "xt": xts[b],
            "w3": np.ascontiguousarray(w3b),
            "b3": np.ascontiguousarray(b3.T),
            "wo": np.ascontiguousarray(wob),
            "cst": cst,
            "vni": vni,
        })
    return in_maps


class _Runner:
    """Cached shard_map runner for the SPMD NEFF (avoids re-jit per call)."""

    def __init__(self, nc):
        import jax
        from jax.sharding import Mesh, PartitionSpec
        from jax.experimental.shard_map import shard_map
        from concourse import bass2jax

        bass2jax.install_neuronx_cc_hook()

        part_name = (nc.partition_id_tensor.name
                     if nc.partition_id_tensor else None)
        in_names, out_names, out_avals, zero_outs = [], [], [], []
        for alloc in nc.m.functions[0].allocations:
            if not isinstance(alloc, mybir.MemoryLocationSet):
                continue
            name = alloc.memorylocations[0].name
            if alloc.kind == "ExternalInput":
                if name != part_name:
                    in_names.append(name)
            elif alloc.kind == "ExternalOutput":
                out_names.append(name)
                shape = tuple(alloc.tensor_shape)
                dtype = mybir.dt.np(alloc.dtype)
                out_avals.append(jax.core.ShapedArray(shape, dtype))
                zero_outs.append(np.zeros(shape, dtype))
        self.in_names, self.out_names = in_names, out_names
        self.zero_outs = zero_outs
        n_params, n_outs = len(in_names), len(out_names)
        all_in_names = tuple(in_names) + tuple(out_names)
        if part_name is not None:
            all_in_names = all_in_names + (part_name,)

        def _exec(args, outs):
            operands = list(args) + list(outs)
            if part_name is not None:
                operands.append(bass2jax.partition_id_tensor())
            return bass2jax._bass_exec_p.bind(
                *operands,
                out_avals=tuple(out_avals),
                in_names=all_in_names,
                out_names=tuple(out_names),
                lowering_input_output_aliases=(),
                sim_require_finite=True,
                sim_require_nnan=True,
                nc=nc)

        def _body(*args):
            ins, outs = args[:n_params], list(args[n_params:])
            return tuple(_exec(ins, outs))

        devices = jax.devices()[:N_CORES]
        mesh = Mesh(np.asarray(devices), ("core",))
        donate = tuple(range(n_params, n_params + n_outs))
        self._fn = jax.jit(
            shard_map(_body, mesh=mesh,
                      in_specs=(PartitionSpec("core"),) * (n_params + n_outs),
                      out_specs=(PartitionSpec("core"),) * n_outs,
                      check_rep=False),
            donate_argnums=donate, keep_unused=True)

    def __call__(self, in_maps):
        concat_in = [
            np.concatenate([in_maps[c][k] for c in range(N_CORES)], axis=0)
            for k in self.in_names]
        concat_zero = [
            np.zeros((N_CORES * z.shape[0], *z.shape[1:]), z.dtype)
            for z in self.zero_outs]
        out = self._fn(*concat_in, *concat_zero)
        return [
            {k: np.asarray(out[i]).reshape(N_CORES, *self.zero_outs[i].shape)[c]
             for i, k in enumerate(self.out_names)}
            for c in range(N_CORES)]


_RUNNERS = {}


def _get_runner(causal: bool):
    if causal not in _RUNNERS:
        _RUNNERS[causal] = _Runner(_get_prog(causal))
    return _RUNNERS[causal]


def kernel(x, Wqkv, bqkv, Wo, bo, mask):
    causal = bool(np.asarray(mask).item()) if not isinstance(mask, (int, bool)) \
        else bool(mask)
    runner = _get_runner(causal)
    in_maps = _prep_inputs(x, Wqkv, bqkv, Wo)
    results = runner(in_maps)
    out = np.zeros((B, T, C), dtype=np.float32)
    for c in range(N_CORES):
        out[c // 4] += results[c]["part"].astype(np.float32)
    out += np.asarray(bo, dtype=np.float32)[None, None, :]
    return out
